# revision 18
# baseline (speedup 1.0000x reference)
"""GPT2 eager causal attention (B=2, S=2048, D=1024, H=16, HD=64) on 8 TRN2 NeuronCores.

Sharding (data + head/tensor parallel, per the problem's hint):
  core c -> (batch b = c//4, head-group g = c%4) -- 4 heads per group.

Per-core pipeline (all layouts chosen so no score-matrix transpose is ever needed):
  1. x[b] transposed on PE -> xT [d, s]                      (d on partitions)
  2. QT,KT = wq/wk^T @ xT  -> [256, S] transposed layouts    (head-dim on partitions)
     V     = xT^T @ wv_ext -> [S, 260] natural, with a ones-column per head
  3. scores^T tiles ST[k, q] = KT_h^T-slices @ QT_h-slices   (k on partitions)
     exp on ScalarE with the 1/sqrt(64) scale folded in; causal masking via
     precomputed mask tiles on diagonal blocks only
     OT[d, q] += V^T-slices @ ST_exp : the ones-column makes row 64 the softmax
     denominator for free; normalize OT by its reciprocal (gpsimd broadcast)
  4. c_proj partial = OT^T-slices @ w_proj[group rows]
  5. ReduceScatter(add) over each 4-core (same-batch) group; each core emits its
     [512, 1024] token slice; host reassembles the [2, 2048, 1024] output.

Matmuls run as float32r (full-rate on PE for free dim >= 256, ~tf32 precision),
fp32 accumulation in PSUM, all storage fp32.
"""
from contextlib import ExitStack

import ml_dtypes
import numpy as np

import concourse.bacc as bacc
import concourse.mybir as mybir
import concourse.tile as tile
from concourse.bass_utils import run_bass_kernel_spmd

F32 = mybir.dt.float32
F32R = mybir.dt.float32r
BF16 = mybir.dt.bfloat16

B, S, D, H, HD = 2, 2048, 1024, 16, 64
N_CORES = 8
HG = 4               # heads per group
DG = HG * HD         # 256 q/k channels per group
VW = HG * (HD + 1)   # 260: 64 v-cols + 1 ones-col per head
NK = D // 128        # 8 contraction tiles over d
NS = S // 128        # 16 token tiles
CH = 512             # q-chunk (one PSUM bank of fp32)
NCH = S // CH        # 4
NRT = DG // 128      # 2 channel row-tiles per group


def _build(has_bv: bool, has_bp: bool, has_bqk: bool = False, tail: str = "rs", phases: int = 99):
    nc = bacc.Bacc("TRN2", target_bir_lowering=False, debug=False, num_devices=N_CORES)

    x_d = nc.dram_tensor("x", [S, D], BF16, kind="ExternalInput").ap()
    wq_d = nc.dram_tensor("wq", [D, DG], BF16, kind="ExternalInput").ap()
    wk_d = nc.dram_tensor("wk", [D, DG], BF16, kind="ExternalInput").ap()
    wv_d = nc.dram_tensor("wv", [D, VW], BF16, kind="ExternalInput").ap()
    wp_d = nc.dram_tensor("wp", [DG, D], BF16, kind="ExternalInput").ap()
    bq_d = nc.dram_tensor("bq", [DG, 1], F32, kind="ExternalInput").ap()
    bk_d = nc.dram_tensor("bk", [DG, 1], F32, kind="ExternalInput").ap()
    bv_d = nc.dram_tensor("bv", [DG, 1], F32, kind="ExternalInput").ap()
    bp_d = nc.dram_tensor("bp", [128, D], F32, kind="ExternalInput").ap()
    mk_d = nc.dram_tensor("masks", [128, 128], BF16, kind="ExternalInput").ap()
    if tail == "rs":
        out_d = nc.dram_tensor("out", [CH, D], F32, kind="ExternalOutput").ap()
    else:  # debug: emit the full per-core partial
        out_d = nc.dram_tensor("out", [S, D], F32, kind="ExternalOutput").ap()

    EXP = mybir.ActivationFunctionType.Exp
    IDENT = mybir.ActivationFunctionType.Identity

    with ExitStack() as ctx:
        tc = ctx.enter_context(tile.TileContext(nc))
        wpool = ctx.enter_context(tc.tile_pool(name="w", bufs=1))
        big = ctx.enter_context(tc.tile_pool(name="big", bufs=8))
        qkvp = ctx.enter_context(tc.tile_pool(name="qkv", bufs=1))
        stp = ctx.enter_context(tc.tile_pool(name="stx", bufs=6))
        nrm = ctx.enter_context(tc.tile_pool(name="nrm", bufs=2))
        outp = ctx.enter_context(tc.tile_pool(name="outp", bufs=3))
        ps_mm = ctx.enter_context(tc.tile_pool(name="psmm", bufs=3, space="PSUM"))
        ps_st = ctx.enter_context(tc.tile_pool(name="psst", bufs=3, space="PSUM"))
        ps_ot = ctx.enter_context(tc.tile_pool(name="psot", bufs=2, space="PSUM"))
        dram = ctx.enter_context(tc.tile_pool(name="dram", bufs=1, space="DRAM"))

        # ---- constants / weights -> SBUF
        wq_sb = wpool.tile([128, NK * DG], BF16)
        wk_sb = wpool.tile([128, NK * DG], BF16)
        wv_sb = wpool.tile([128, NK * VW], BF16)
        wp_sb = wpool.tile([128, NRT * D], BF16)
        mk_sb = wpool.tile([128, 128], BF16)
        on_sb = wpool.tile([1, 64], F32)
        bq_sb = wpool.tile([128, NRT], F32)
        bk_sb = wpool.tile([128, NRT], F32)
        bv_sb = wpool.tile([128, NRT], F32) if has_bv else None
        bp_sb = wpool.tile([128, D], F32) if has_bp else None
        for kt in range(NK):
            nc.sync.dma_start(wq_sb[:, kt * DG:(kt + 1) * DG], wq_d[kt * 128:(kt + 1) * 128, :])
            nc.sync.dma_start(wk_sb[:, kt * DG:(kt + 1) * DG], wk_d[kt * 128:(kt + 1) * 128, :])
            nc.sync.dma_start(wv_sb[:, kt * VW:(kt + 1) * VW], wv_d[kt * 128:(kt + 1) * 128, :])
        for rt in range(NRT):
            nc.sync.dma_start(wp_sb[:, rt * D:(rt + 1) * D], wp_d[rt * 128:(rt + 1) * 128, :])
            nc.sync.dma_start(bq_sb[:, rt:rt + 1], bq_d[rt * 128:(rt + 1) * 128, :])
            nc.sync.dma_start(bk_sb[:, rt:rt + 1], bk_d[rt * 128:(rt + 1) * 128, :])
            if has_bv:
                nc.sync.dma_start(bv_sb[:, rt:rt + 1], bv_d[rt * 128:(rt + 1) * 128, :])
        if has_bp:
            nc.sync.dma_start(bp_sb[:], bp_d[:])
        nc.sync.dma_start(mk_sb[:], mk_d[:])
        nc.vector.memset(on_sb[:], 1.0)

        # ---- phase 1: xT strips [128 d, S] via transpose-DMA (bf16 xbar path),
        # split into column chunks; weights were enqueued first so QKV can
        # start as soon as the sq=0 chunks land
        xT = []
        for dt in range(NK):
            t = big.tile([128, S], BF16, tag="bigslot", name=f"xT{dt}")
            xT.append(t)
        for sq in range(4):
            for dt in range(NK):
                nc.sync.dma_start_transpose(
                    xT[dt][:, sq * CH:(sq + 1) * CH],
                    x_d[sq * CH:(sq + 1) * CH, dt * 128:(dt + 1) * 128],
                )


        # ---- phase 2: QT/KT [256, S] (as 2 tiles of [128, S]) and V strips
        QT, KT = [], []
        for store, w_sb, b_sb, nm in ((QT, wq_sb, bq_sb, "q"), (KT, wk_sb, bk_sb, "k")):
            for rt in range(NRT):
                dst = qkvp.tile([128, S], BF16, tag=f"{nm}t{rt}", name=f"{nm}T{rt}")
                store.append(dst)
                for ch in range(NCH):
                    ps = ps_mm.tile([128, CH], F32, tag="ps", name=f"ps{nm}{rt}_{ch}")
                    for kt in range(NK):
                        nc.tensor.matmul(
                            ps[:],
                            (w_sb[:, kt * DG + rt * 128: kt * DG + (rt + 1) * 128]),
                            (xT[kt][:, ch * CH:(ch + 1) * CH]),
                            start=(kt == 0), stop=(kt == NK - 1),
                        )
                    if has_bqk:
                        nc.scalar.activation(
                            dst[:, ch * CH:(ch + 1) * CH], ps[:], IDENT,
                            bias=b_sb[:, rt:rt + 1],
                        )
                    else:
                        nc.vector.tensor_copy(dst[:, ch * CH:(ch + 1) * CH], ps[:])
        V = []
        for st in range(NS):
            vt = qkvp.tile([128, VW], BF16, tag=f"v{st}", name=f"v{st}")
            ps = ps_mm.tile([128, CH], F32, tag="ps", name=f"psv{st}")
            for kt in range(NK):
                nc.tensor.matmul(
                    ps[:, :VW],
                    (xT[kt][:, st * 128:(st + 1) * 128]),
                    (wv_sb[:, kt * VW:(kt + 1) * VW]),
                    start=(kt == 0), stop=(kt == NK - 1),
                )
            nc.vector.tensor_copy(vt[:], ps[:, :VW])
            for hl in range(HG):
                ones_col = vt[:, hl * (HD + 1) + HD: (hl + 1) * (HD + 1)].bitcast(mybir.dt.uint16)
                nc.vector.memset(ones_col, 0x3F80)  # bits of bf16 1.0
            V.append(vt)

        # ---- phase 3: attention + c_proj, chunk by chunk
        OT = []
        for i in range(NRT):
            t = big.tile([128, S], BF16, tag="bigslot", name=f"OT{i}")
            OT.append(t)
        partials = []
        for ch in range(NCH):
            pt = dram.tile([CH, D], F32, tag=f"partial{ch}", name=f"partial{ch}")
            partials.append(pt)
        rs_outs = []

        def emit_rs(ch):
            # rank r of the quad receives tokens [512*ch + 128*r, +128)
            rs_c = dram.tile([128, D], F32, tag=f"rs{ch}", name=f"rs_out{ch}")
            nc.gpsimd.collective_compute(
                "ReduceScatter",
                mybir.AluOpType.add,
                replica_groups=[[0, 1, 2, 3], [4, 5, 6, 7]],
                ins=[partials[ch].opt()],
                outs=[rs_c.opt()],
            )
            rs_outs.append((ch, rs_c))
        for ch in range(NCH):
            nkt = 4 * (ch + 1)
            for hl in range(HG):
                qt = QT[hl // 2]
                ktile = KT[hl // 2]
                off = 64 * (hl % 2)
                ot_ps = ps_ot.tile([65, CH], F32, tag="ot", name=f"ot{ch}_{hl}")
                for kt in range(nkt):
                    st_ps = ps_st.tile([128, CH], F32, tag="st", name=f"st{ch}_{hl}_{kt}")
                    nc.tensor.matmul(
                        st_ps[:],
                        (ktile[off:off + 64, kt * 128:(kt + 1) * 128]),
                        (qt[off:off + 64, ch * CH:(ch + 1) * CH]),
                        start=True, stop=True,
                    )
                    st_sb = stp.tile([128, CH], BF16, tag="stsb", name=f"se{ch}_{hl}_{kt}")
                    d = kt - 4 * ch
                    if d < 0:
                        nc.scalar.activation(st_sb[:], st_ps[:], EXP, scale=0.125)
                    else:
                        # diagonal strip: exp only the valid suffix, zero the
                        # prefix, triangular-mask the 128-wide diagonal block
                        if d > 0:
                            zc = st_sb[:, 0:d * 128].bitcast(mybir.dt.uint16)
                            nc.vector.memset(zc, 0)
                        nc.scalar.activation(st_sb[:, d * 128:], st_ps[:, d * 128:], EXP, scale=0.125)
                        nc.vector.tensor_mul(
                            st_sb[:, d * 128:(d + 1) * 128],
                            st_sb[:, d * 128:(d + 1) * 128],
                            mk_sb[:, 0:128],
                        )
                    nc.tensor.matmul(
                        ot_ps[:],
                        (V[kt][:, hl * (HD + 1):(hl + 1) * (HD + 1)]),
                        (st_sb[:]),
                        start=(kt == 0), stop=(kt == nkt - 1),
                    )
                den = nrm.tile([1, CH], F32, tag="den", name=f"den{ch}_{hl}")
                nc.vector.tensor_copy(den[:], ot_ps[64:65, :])
                rden = nrm.tile([1, CH], F32, tag="rden", name=f"rden{ch}_{hl}")
                nc.vector.reciprocal_approx_fast(rden[:], den[:])
                # rank-1 PE matmul broadcasts the reciprocal row to 64
                # partitions (keeps gpsimd free for the collectives)
                rbc_ps = ps_mm.tile([64, CH], F32, tag="ps", name=f"rbc{ch}_{hl}")
                nc.tensor.matmul(rbc_ps[:], on_sb[:], rden[:], start=True, stop=True)
                ot_sb = nrm.tile([64, CH], BF16, tag="otsb", name=f"otsb{ch}_{hl}")
                nc.vector.tensor_copy(ot_sb[:], ot_ps[0:64, :])
                dst = OT[hl // 2][off:off + 64, ch * CH:(ch + 1) * CH]
                nc.vector.tensor_mul(dst, ot_sb[:], rbc_ps[:])
                if has_bv:
                    nc.vector.tensor_scalar_add(dst, dst, bv_sb[off:off + 64, hl // 2: hl // 2 + 1])
            # c_proj for this chunk's tokens
            for stl in range(4):
                tok = ch * CH + stl * 128
                for n in range(NRT):
                    po = ps_mm.tile([128, CH], F32, tag="ps", name=f"po{ch}_{stl}_{n}")
                    for k2 in range(NRT):
                        nc.tensor.matmul(
                            po[:],
                            (OT[k2][:, tok:tok + 128]),
                            (wp_sb[:, k2 * D + n * CH: k2 * D + (n + 1) * CH]),
                            start=(k2 == 0), stop=(k2 == NRT - 1),
                        )
                    ob = outp.tile([128, CH], F32, tag="ob", name=f"ob{ch}_{stl}_{n}")
                    if has_bp:
                        nc.vector.tensor_add(ob[:], po[:], bp_sb[:, n * CH:(n + 1) * CH])
                    else:
                        nc.vector.tensor_copy(ob[:], po[:])
                    if tail == "rs":
                        nc.sync.dma_start(partials[ch][stl * 128:(stl + 1) * 128, n * CH:(n + 1) * CH], ob[:])
                    else:
                        nc.sync.dma_start(out_d[tok:tok + 128, n * CH:(n + 1) * CH], ob[:])
            if tail == "rs":
                emit_rs(ch)
        if tail == "rs":
            # final out DMAs last: keeps the in-order sync queue from blocking
            # mid-kernel partial writes behind collective completion waits
            for ch, rs_c in rs_outs:
                nc.sync.dma_start(out_d[ch * 128:(ch + 1) * 128, :], rs_c[:])

    nc.compile()
    return nc


_prog_cache = {}


def _get_prog(has_bv, has_bp, has_bqk):
    key = (has_bv, has_bp, has_bqk)
    if key not in _prog_cache:
        _prog_cache[key] = _build(has_bv, has_bp, has_bqk)
    return _prog_cache[key]


def _prepare(x, w_attn, b_attn, w_proj, b_proj):
    x = np.asarray(x, dtype=np.float32)
    w_attn = np.asarray(w_attn, dtype=np.float32)
    b_attn = np.asarray(b_attn, dtype=np.float32)
    w_proj = np.asarray(w_proj, dtype=np.float32)
    b_proj = np.asarray(b_proj, dtype=np.float32)

    has_bv = bool(np.any(b_attn[2 * D:]))
    has_bp = bool(np.any(b_proj))
    has_bqk = bool(np.any(b_attn[:2 * D]))
    nc = _get_prog(has_bv, has_bp, has_bqk)

    ii = np.arange(128)[:, None]
    jj = np.arange(128)[None, :]
    masks = (jj >= ii).astype(np.float32).astype(ml_dtypes.bfloat16)

    in_maps = []
    for c in range(N_CORES):
        b, g = divmod(c, 4)
        q0 = g * DG
        k0 = D + g * DG
        v0 = 2 * D + g * DG
        wv_ext = np.zeros((D, VW), dtype=np.float32)
        for hl in range(HG):
            wv_ext[:, hl * (HD + 1):hl * (HD + 1) + HD] = w_attn[:, v0 + hl * HD: v0 + (hl + 1) * HD]
        if g == 0:
            bp_tile = np.broadcast_to(b_proj, (128, D)).astype(np.float32)
        else:
            bp_tile = np.zeros((128, D), dtype=np.float32)
        in_maps.append({
            "x": np.ascontiguousarray(x[b]).astype(ml_dtypes.bfloat16),
            "wq": np.ascontiguousarray(w_attn[:, q0:q0 + DG]).astype(ml_dtypes.bfloat16),
            "wk": np.ascontiguousarray(w_attn[:, k0:k0 + DG]).astype(ml_dtypes.bfloat16),
            "wv": wv_ext.astype(ml_dtypes.bfloat16),
            "wp": np.ascontiguousarray(w_proj[g * DG:(g + 1) * DG, :]).astype(ml_dtypes.bfloat16),
            "bq": np.ascontiguousarray(b_attn[q0:q0 + DG, None]),
            "bk": np.ascontiguousarray(b_attn[k0:k0 + DG, None]),
            "bv": np.ascontiguousarray(b_attn[v0:v0 + DG, None]),
            "bp": bp_tile,
            "masks": masks,
        })
    return nc, in_maps


def _assemble(results):
    out = np.empty((B, S, D), dtype=np.float32)
    for c in range(N_CORES):
        b, g = divmod(c, 4)
        o = results[c]["out"]
        for ch in range(NCH):
            tok = ch * CH + g * 128
            out[b, tok:tok + 128, :] = o[ch * 128:(ch + 1) * 128, :]
    return out


def kernel(x, w_attn, b_attn, w_proj, b_proj):
    nc, in_maps = _prepare(x, w_attn, b_attn, w_proj, b_proj)
    res = run_bass_kernel_spmd(nc, in_maps, list(range(N_CORES)))
    return _assemble(res.results)
